# revision 2
# baseline (speedup 1.0000x reference)
"""Trainium2 Bass kernel for nn_L4maAttention (llama3.1-style GQA attention layer).

Sharding: heads across 8 cores (4 Q heads + 1 KV head per core).
  - q/k/v projections column-parallel, rope on device
  - paged-KV context gathered on host, shipped pre-transposed per core
  - attention computed per-head locally in S^T layout ([kv, q]) so the
    softmax'd P tile is directly the moving operand of the P@V matmul
  - denominators via an all-ones stationary matmul accumulated in PSUM
  - o_proj row-parallel; per-core partial outputs summed on host

All matmuls run as float32r (TF32-like, 1 cycle/row for moving dim >= 256).
"""

import math
import os
import sys

import numpy as np

sys.path.insert(0, "/opt/trn_rl_repo")

import concourse.bass as bass  # noqa: E402
import concourse.mybir as mybir  # noqa: E402
import concourse.tile as tile  # noqa: E402
from concourse import bacc  # noqa: E402
from concourse.bass_utils import run_bass_kernel_spmd  # noqa: E402
from concourse.masks import make_identity  # noqa: E402

# ---- problem constants (hardcoded from spec) ----
B, QO, PAGE = 4, 512, 16
HID, HQ, HKV, D = 4096, 32, 8, 128
N = B * QO  # 2048
NCORES = 8
HQL = HQ // NCORES  # 4 local q heads
ROPE_THETA = 5e5
OLD_CTX, LOW_F, HIGH_F, RSCALE = 8192.0, 1.0, 4.0, 8.0
SM_SCALE = 1.0 / math.sqrt(D)

import ml_dtypes
BF16NP = ml_dtypes.bfloat16
F32 = mybir.dt.float32
BF16 = mybir.dt.bfloat16
F32R = mybir.dt.float32r
AF = mybir.ActivationFunctionType
ALU = mybir.AluOpType
P = 128


def _llama31_inv_freq(d):
    inv = ROPE_THETA ** (-np.arange(0, d, 2, dtype=np.float32) / d)
    wavelen = 2.0 * np.pi / inv
    low_wl, high_wl = OLD_CTX / LOW_F, OLD_CTX / HIGH_F
    smooth = (OLD_CTX / wavelen - LOW_F) / (HIGH_F - LOW_F)
    mid = (1.0 - smooth) * inv / RSCALE + smooth * inv
    return np.where(
        wavelen > low_wl, inv / RSCALE, np.where(wavelen < high_wl, inv, mid)
    ).astype(np.float32)


def host_prep(inputs):
    """Shard + pre-transpose inputs for the 8 cores. Returns (in_maps, ctxl)."""
    hs = np.ascontiguousarray(np.asarray(inputs["hidden_states"], np.float32))
    pos_ids = np.asarray(inputs["position_ids"], np.int32)
    kvc = np.asarray(inputs["kv_cache"], np.float32)
    kpi = np.asarray(inputs["kv_page_indices"], np.int32)
    kpp = np.asarray(inputs["kv_page_indptr"], np.int32)
    klp = np.asarray(inputs["kv_last_page_lens"], np.int32)
    qop = np.asarray(inputs["qo_indptr"], np.int32)
    Wq = np.asarray(inputs["Wq"], np.float32)
    Wk = np.asarray(inputs["Wk"], np.float32)
    Wv = np.asarray(inputs["Wv"], np.float32)
    Wo = np.asarray(inputs["Wo"], np.float32)

    n, hid = hs.shape
    b_sz = qop.shape[0] - 1
    qo_len = n // b_sz
    page = kvc.shape[2]
    pps = kpi.shape[0] // b_sz
    seq_len = (pps - 1) * page + klp  # [B]
    ctx_len = seq_len - qo_len
    assert n == N and hid == HID and b_sz == B and qo_len == QO
    assert np.all(ctx_len == ctx_len[0]) and int(ctx_len[0]) % 128 == 0
    ctxl = int(ctx_len[0])

    # rope tables [64, N] indexed (freq, token)
    inv = _llama31_inv_freq(D)
    ang = pos_ids.astype(np.float32)[:, None] * inv[None, :]
    cosT = np.ascontiguousarray(np.cos(ang).T).astype(np.float32)
    sinT = np.ascontiguousarray(np.sin(ang).T).astype(np.float32)

    # gather paged KV context (positions 0..ctxl-1 per sequence)
    cpos = np.arange(ctxl)
    pages = kpi[kpp[:-1][:, None] + (cpos[None, :] // page)]  # [B, ctxl]
    slots = np.broadcast_to(cpos % page, (b_sz, ctxl))
    Kc = kvc[pages, 0, slots]  # [B, ctxl, HKV, D]
    Vc = kvc[pages, 1, slots]

    # causal mask for the new-kv block, tiled [128, 4*512]: chunk c holds
    # rows kv_rel in [c*128,(c+1)*128) vs all 512 q_rel columns
    qr = np.arange(qo_len)
    mbig = np.where(qr[:, None] <= qr[None, :], 0.0, -1e30).astype(np.float32)
    msk = np.ascontiguousarray(
        np.concatenate([mbig[i * 128 : (i + 1) * 128] for i in range(qo_len // 128)], axis=1)
    )
    ones = np.ones((P, P), BF16NP)
    hT = np.ascontiguousarray(hs.T).astype(BF16NP)

    Wq4 = Wq.reshape(HQ, D, HID)
    Wk4 = Wk.reshape(HKV, D, HID)
    Wv4 = Wv.reshape(HKV, D, HID)
    Wo4 = Wo.reshape(HID, HQ, D)

    in_maps = []
    for i in range(NCORES):
        wqT = np.ascontiguousarray(Wq4[i * HQL : (i + 1) * HQL].reshape(HQL * D, HID).T).astype(BF16NP)
        wkT = np.ascontiguousarray(Wk4[i].T).astype(BF16NP)
        wvT = np.ascontiguousarray(Wv4[i].T).astype(BF16NP)
        woT = np.ascontiguousarray(Wo4[:, i * HQL : (i + 1) * HQL, :].reshape(HID, HQL * D).T).astype(BF16NP)
        kctxT = np.ascontiguousarray(Kc[:, :, i, :].reshape(b_sz * ctxl, D).T).astype(BF16NP)
        vctx = np.ascontiguousarray(
            Vc[:, :, i, :].reshape(-1, 128, D).transpose(1, 0, 2).reshape(128, b_sz * ctxl)
        ).astype(BF16NP)
        in_maps.append(
            dict(hT=hT, wqT=wqT, wkT=wkT, wvT=wvT, woT=woT, kctxT=kctxT,
                 vctx=vctx, cosT=cosT, sinT=sinT, msk=msk, ones=ones)
        )
    return in_maps, ctxl


def _rope_evict(nc, tpool, psum, dst, cs, sn):
    """dst[0:64] = p1*cos - p2*sin ; dst[64:128] = p2*cos + p1*sin."""
    t1 = tpool.tile([64, 512], F32, tag="t1")
    t2 = tpool.tile([64, 512], F32, tag="t2")
    t3 = tpool.tile([64, 512], F32, tag="t3")
    t4 = tpool.tile([64, 512], F32, tag="t4")
    nc.vector.tensor_tensor(t1[:], psum[0:64, :], cs, ALU.mult)
    nc.vector.tensor_tensor(t2[:], psum[64:128, :], sn, ALU.mult)
    nc.vector.tensor_tensor(dst[0:64, :], t1[:], t2[:], ALU.subtract)
    nc.vector.tensor_tensor(t3[:], psum[64:128, :], cs, ALU.mult)
    nc.vector.tensor_tensor(t4[:], psum[0:64, :], sn, ALU.mult)
    nc.vector.tensor_tensor(dst[64:128, :], t3[:], t4[:], ALU.add)


def build_program(ctxl):
    KVL = ctxl + QO  # kv length per sequence
    CC = ctxl // 128  # context chunks per sequence
    KC = KVL // 128  # total kv chunks per sequence
    NT = N // 512  # token chunks of 512 (== B)
    KH = HID // 128  # contraction chunks for projections

    nc = bacc.Bacc("TRN2", debug=False, num_devices=NCORES)
    hT = nc.dram_tensor("hT", [HID, N], BF16, kind="ExternalInput").ap()
    wqT = nc.dram_tensor("wqT", [HID, HQL * D], BF16, kind="ExternalInput").ap()
    wkT = nc.dram_tensor("wkT", [HID, D], BF16, kind="ExternalInput").ap()
    wvT = nc.dram_tensor("wvT", [HID, D], BF16, kind="ExternalInput").ap()
    woT = nc.dram_tensor("woT", [HQL * D, HID], BF16, kind="ExternalInput").ap()
    kctxT = nc.dram_tensor("kctxT", [D, B * ctxl], BF16, kind="ExternalInput").ap()
    vctx = nc.dram_tensor("vctx", [P, B * ctxl], BF16, kind="ExternalInput").ap()
    cosT = nc.dram_tensor("cosT", [D // 2, N], F32, kind="ExternalInput").ap()
    sinT = nc.dram_tensor("sinT", [D // 2, N], F32, kind="ExternalInput").ap()
    msk = nc.dram_tensor("msk", [P, (QO // 128) * QO], F32, kind="ExternalInput").ap()
    ones = nc.dram_tensor("ones", [P, P], BF16, kind="ExternalInput").ap()
    out = nc.dram_tensor("out", [N, HID], F32, kind="ExternalOutput").ap()

    with tile.TileContext(nc) as tc:
        with tc.tile_pool(name="resident", bufs=1) as res:
            q_sb = res.tile([P, HQL * N], BF16)  # head h at cols [h*N, (h+1)*N)
            kn_sb = res.tile([P, N], BF16)  # new K^T, batch b at cols b*512
            vn_sb = res.tile([P, N], BF16)  # new V, chunk t=(b*4+j) at cols t*128
            o_sb = res.tile([P, 16 * 512], BF16)  # O^T, (b,h) at cols (b*4+h)*512
            cos_sb = res.tile([D // 2, N], F32)
            sin_sb = res.tile([D // 2, N], F32)
            ones_sb = res.tile([P, P], BF16)
            ident = res.tile([P, P], BF16)
            nc.sync.dma_start(cos_sb[:], cosT)
            nc.sync.dma_start(sin_sb[:], sinT)
            nc.sync.dma_start(ones_sb[:], ones)
            make_identity(nc, ident[:])

            # ================= Phase A: QKV projections + rope =================
            with tc.tile_pool(name="wsb", bufs=1) as wpool, \
                 tc.tile_pool(name="wqstream", bufs=4) as wqpool, \
                 tc.tile_pool(name="hstream", bufs=4) as hpool, \
                 tc.tile_pool(name="qkvpsum", bufs=1, space="PSUM") as ppool, \
                 tc.tile_pool(name="vtpsum", bufs=2, space="PSUM") as vtpool, \
                 tc.tile_pool(name="ropetmp", bufs=2) as tpool, \
                 tc.tile_pool(name="vsb", bufs=2) as vsbpool:
                wk_sb = wpool.tile([P, KH * D], BF16)
                wv_sb = wpool.tile([P, KH * D], BF16)
                for k in range(KH):
                    nc.sync.dma_start(wk_sb[:, k * 128 : (k + 1) * 128],
                                      wkT[k * 128 : (k + 1) * 128, :])
                    nc.sync.dma_start(wv_sb[:, k * 128 : (k + 1) * 128],
                                      wvT[k * 128 : (k + 1) * 128, :])
                for n in range(NT):
                    ps = [ppool.tile([P, 512], F32, tag=f"m{m}", name=f"ps_{n}_{m}")
                          for m in range(6)]
                    for k in range(KH):
                        ht = hpool.tile([P, 512], BF16)
                        nc.sync.dma_start(ht[:], hT[k * 128 : (k + 1) * 128,
                                                    n * 512 : (n + 1) * 512])
                        wqt = wqpool.tile([P, 512], BF16)
                        nc.sync.dma_start(wqt[:], wqT[k * 128 : (k + 1) * 128, :])
                        rhs = ht[:]
                        st, sp = (k == 0), (k == KH - 1)
                        for m in range(HQL):
                            nc.tensor.matmul(
                                ps[m][:],
                                wqt[:, m * 128 : (m + 1) * 128],
                                rhs, start=st, stop=sp)
                        nc.tensor.matmul(
                            ps[4][:], wk_sb[:, k * 128 : (k + 1) * 128],
                            rhs, start=st, stop=sp)
                        nc.tensor.matmul(
                            ps[5][:], wv_sb[:, k * 128 : (k + 1) * 128],
                            rhs, start=st, stop=sp)
                    cs = cos_sb[:, n * 512 : (n + 1) * 512]
                    sn = sin_sb[:, n * 512 : (n + 1) * 512]
                    for m in range(HQL):
                        _rope_evict(nc, tpool, ps[m],
                                    q_sb[:, m * N + n * 512 : m * N + (n + 1) * 512],
                                    cs, sn)
                    _rope_evict(nc, tpool, ps[4],
                                kn_sb[:, n * 512 : (n + 1) * 512], cs, sn)
                    vt = vsbpool.tile([P, 512], BF16)
                    nc.scalar.activation(vt[:], ps[5][:], AF.Copy)
                    for j in range(4):
                        tp = vtpool.tile([P, P], BF16)
                        nc.tensor.transpose(tp[:], vt[:, j * 128 : (j + 1) * 128], ident[:])
                        nc.scalar.activation(
                            vn_sb[:, (n * 4 + j) * 128 : (n * 4 + j + 1) * 128],
                            tp[:], AF.Copy)

            # ================= Phase B: attention =================
            with tc.tile_pool(name="kvsb", bufs=1) as kvpool, \
                 tc.tile_pool(name="spsum", bufs=2, space="PSUM") as spool, \
                 tc.tile_pool(name="opsum", bufs=2, space="PSUM") as opool, \
                 tc.tile_pool(name="dpsum", bufs=2, space="PSUM") as dpool, \
                 tc.tile_pool(name="ptile", bufs=3) as p2pool, \
                 tc.tile_pool(name="rtile", bufs=2) as rpool:
                kctx_sb = kvpool.tile([P, B * ctxl], BF16)
                vctx_sb = kvpool.tile([P, B * ctxl], BF16)
                msk_sb = kvpool.tile([P, (QO // 128) * QO], F32)
                nc.sync.dma_start(kctx_sb[:], kctxT)
                nc.sync.dma_start(vctx_sb[:], vctx)
                nc.sync.dma_start(msk_sb[:], msk)
                for b in range(B):
                    for h in range(HQL):
                        po = opool.tile([P, 512], F32)
                        pd = dpool.tile([P, 512], F32)
                        qap = q_sb[:, h * N + b * 512 : h * N + (b + 1) * 512]
                        for c in range(KC):
                            if c < CC:
                                kl = kctx_sb[:, b * ctxl + c * 128 : b * ctxl + (c + 1) * 128]
                                vl = vctx_sb[:, b * ctxl + c * 128 : b * ctxl + (c + 1) * 128]
                            else:
                                j = c - CC
                                kl = kn_sb[:, b * 512 + j * 128 : b * 512 + (j + 1) * 128]
                                vl = vn_sb[:, (b * 4 + j) * 128 : (b * 4 + j + 1) * 128]
                            st = spool.tile([P, 512], F32)
                            nc.tensor.matmul(st[:], kl, qap,
                                             start=True, stop=True)
                            if c >= CC:
                                j = c - CC
                                nc.vector.tensor_tensor(
                                    st[:], st[:], msk_sb[:, j * 512 : (j + 1) * 512],
                                    ALU.add)
                            pt = p2pool.tile([P, 512], BF16)
                            nc.scalar.activation(pt[:], st[:], AF.Exp, scale=SM_SCALE)
                            prhs = pt[:]
                            nc.tensor.matmul(po[:], vl, prhs,
                                             start=(c == 0), stop=(c == KC - 1))
                            nc.tensor.matmul(pd[:], ones_sb[:], prhs,
                                             start=(c == 0), stop=(c == KC - 1))
                        dsb = rpool.tile([P, 512], F32)
                        nc.scalar.activation(dsb[:], pd[:], AF.Copy)
                        rsb = rpool.tile([P, 512], F32, tag="rsb")
                        nc.vector.reciprocal(rsb[:], dsb[:])
                        nc.vector.tensor_tensor(
                            o_sb[:, (b * 4 + h) * 512 : (b * 4 + h + 1) * 512],
                            po[:], rsb[:], ALU.mult)

            # ================= Phase C: o_proj (partial) =================
            with tc.tile_pool(name="wostream", bufs=2) as wopool, \
                 tc.tile_pool(name="cpsum", bufs=2, space="PSUM") as cpool, \
                 tc.tile_pool(name="outsb", bufs=3) as outpool:
                for nh in range(HID // 512):
                    wt = wopool.tile([P, HQL * 512], BF16)
                    for h in range(HQL):
                        nc.sync.dma_start(
                            wt[:, h * 512 : (h + 1) * 512],
                            woT[h * 128 : (h + 1) * 128, nh * 512 : (nh + 1) * 512])
                    for t in range(N // 128):
                        b, qs = divmod(t, 4)
                        pc = cpool.tile([P, 512], F32)
                        for h in range(HQL):
                            lhsT = o_sb[:, (b * 4 + h) * 512 + qs * 128 :
                                        (b * 4 + h) * 512 + (qs + 1) * 128]
                            nc.tensor.matmul(pc[:], lhsT,
                                             wt[:, h * 512 : (h + 1) * 512],
                                             start=(h == 0), stop=(h == HQL - 1))
                        ot = outpool.tile([P, 512], F32)
                        nc.scalar.activation(ot[:], pc[:], AF.Copy)
                        nc.sync.dma_start(
                            out[t * 128 : (t + 1) * 128, nh * 512 : (nh + 1) * 512],
                            ot[:])
    nc.compile()
    return nc


_NC_CACHE = {}


def _get_program(ctxl):
    if ctxl not in _NC_CACHE:
        _NC_CACHE[ctxl] = build_program(ctxl)
    return _NC_CACHE[ctxl]


def run(inputs, trace=False):
    in_maps, ctxl = host_prep(inputs)
    nc = _get_program(ctxl)
    kw = dict(tmpdir="/tmp/trace_out") if trace else {}
    res = run_bass_kernel_spmd(nc, in_maps, core_ids=list(range(NCORES)), trace=trace, **kw)
    out = np.zeros((N, HID), np.float32)
    for r in res.results:
        out += np.asarray(r["out"], np.float32)
    return out, res


def kernel(**inputs) -> np.ndarray:
    out, _ = run(inputs, trace=False)
    return out

